# revision 34
# baseline (speedup 1.0000x reference)
"""Differential attention kernel for 8 Trainium2 NeuronCores — v5.

Key-sharded: core c handles batch c//2 and key-half c%2 (2048 keys), all
4096 queries. Per-head partial numerators + row sums go back to the host,
which adds the two halves and normalizes (softmax denominators span both
cores' keys).

fp8 (e4m3) DoubleRow matmuls carry the projections and head-1 PV:
- projections use a 3-term hi/lo split (x and W each split into e4m3
  hi + residual lo; terms hi*hi + hi*lo + lo*hi) — bf16-level accuracy at
  0.75x the bf16 matmul cost.
- head-1 PV uses single-e4m3 exp scores against hi/lo-split V — half the
  bf16 PV cost. Head 2 (amplified by lambda≈2.8) stays bf16.
- scores stay bf16 (contraction is only 128, DoubleRow needs 256).
exp is biased by -3 so e4m3's max (224) covers the largest logit (~8.02).

Schedule notes: score emission is software-pipelined one key-group ahead
so the in-order PE queue keeps a score matmul in flight while the
activation engine runs exp. The next chunk's q-projection blocks and the
next loop's first score group are emitted as tail fillers inside each
head's last key-group (covering the final exp wait), time-sharing the
score-psum slots (PSUM is exactly full: 4 score banks + 4 PV banks).
DMAs are whole-chunk single transfers — the SP engine serializes DMA
dispatch, so x chunks go through the otherwise-idle Pool engine
(software DGE) as a parallel DMA channel.
"""

import math
import os
import time
from contextlib import ExitStack

import ml_dtypes
import numpy as np

import concourse.bass as bass
from concourse import bacc
import concourse.mybir as mybir
import concourse.tile as tile
from concourse.bass_utils import run_bass_kernel_spmd

B, S, D = 4, 4096, 2048
HD = 128
DV = 256
DVA = DV + 1      # + ones column for row sums
SK = S // 2       # keys per core
N_CORES = 8
DEPTH = 12
SCALE = HD ** -0.5

DT = D // 128     # 16 d-tiles
SKT = SK // 128   # 16 key tiles per core
KG = SKT // 2     # 8 key groups (DoubleRow pairs)
QC = S // 512     # 8 query chunks
KC = SK // 512    # 4 key chunks (phase-1 k/v projection)

W_SC = 32.0       # W pre-scale before e4m3 (W entries ~N(0, 1/D))
INV_SC = 1.0 / W_SC
ET_BIAS = -3.0    # exp(a - 3): keeps max exp < e4m3 max (224)

F8 = mybir.dt.float8e4
BF16 = mybir.dt.bfloat16
F32 = mybir.dt.float32
DR = mybir.MatmulPerfMode.DoubleRow
MULT = mybir.AluOpType.mult
SUB = mybir.AluOpType.subtract

_cache = {}


def build_nc():
    nc = bacc.Bacc("TRN2", target_bir_lowering=False, debug=False)

    xh_d = nc.declare_dram_parameter("xh", [D, S], F8, isOutput=False)
    xl_d = nc.declare_dram_parameter("xl", [D, S], F8, isOutput=False)
    wh_d = nc.declare_dram_parameter("wh", [128, 3 * DT * DV], F8, isOutput=False)
    wl_d = nc.declare_dram_parameter("wl", [128, 3 * DT * DV], F8, isOutput=False)
    out_d = nc.declare_dram_parameter("out", [2, S, DVA], BF16, isOutput=True)

    # tiled views: [p, dt, col] so one DMA moves a whole chunk (the SP
    # engine serializes DMAs; per-dt dma_starts would swamp it)
    xh = xh_d.ap().rearrange("(dt p) s -> p dt s", p=128)      # [128, DT, S]
    xl = xl_d.ap().rearrange("(dt p) s -> p dt s", p=128)
    wh = wh_d.ap().rearrange("p (g dt c) -> p g dt c", g=3, dt=DT)  # [128, 3, DT, 256]
    wl = wl_d.ap().rearrange("p (g dt c) -> p g dt c", g=3, dt=DT)
    out = out_d.ap().rearrange("h (qs p) d -> h p qs d", p=128)  # [2, 128, 32, DVA]

    with tile.TileContext(nc) as tc, ExitStack() as ctx:
        singles = ctx.enter_context(tc.tile_pool(name="singles", bufs=1))
        x_pool = ctx.enter_context(tc.tile_pool(name="x", bufs=10))
        e_pool = ctx.enter_context(tc.tile_pool(name="e", bufs=8))
        o_pool = ctx.enter_context(tc.tile_pool(name="o", bufs=6))

        w_hi = singles.tile([128, 3, DT, DV], F8, tag="w_hi")
        w_lo = singles.tile([128, 3, DT, DV], F8, tag="w_lo")
        kT = singles.tile([128, 2, SK], BF16, tag="kT")       # [dh, head, key]
        qT = singles.tile([128, 2, S], BF16, tag="qT")        # [dh, head, query]
        vhi = singles.tile([128, SKT, DVA], F8, tag="vhi")    # [key_row, kt, dv+1]
        vlo = singles.tile([128, SKT, DVA], F8, tag="vlo")
        vaug = singles.tile([128, SKT, DVA], BF16, tag="vaug")
        bias_t = singles.tile([128, 1], F32, tag="bias")

        nc.vector.memset(bias_t, ET_BIAS)
        nc.vector.memset(vhi[:, :, DV:DVA], 1.0)
        nc.vector.memset(vlo[:, :, DV:DVA], 0.0)
        nc.vector.memset(vaug[:, :, DV:DVA], 1.0)

        chunks = {}

        def dma_chunk(sc):
            th = x_pool.tile([128, DT, 512], F8, tag="xt", name=f"xh_{sc}")
            tl = x_pool.tile([128, DT, 512], F8, tag="xt", name=f"xl_{sc}")
            nc.gpsimd.dma_start(out=th, in_=xh[:, :, sc * 512:(sc + 1) * 512])
            nc.sync.dma_start(out=tl, in_=xl[:, :, sc * 512:(sc + 1) * 512])
            chunks[sc] = (th, tl)

        # K weights first, then chunk 0, V weights, chunk 1, Q weights last
        # (q-projection happens in phase 2) — SP runs DMAs strictly in order.
        for g in (1,):
            nc.sync.dma_start(out=w_hi[:, g], in_=wh[:, g])
            nc.sync.dma_start(out=w_lo[:, g], in_=wl[:, g])
        dma_chunk(0)
        for g in (2,):
            nc.sync.dma_start(out=w_hi[:, g], in_=wh[:, g])
            nc.sync.dma_start(out=w_lo[:, g], in_=wl[:, g])
        dma_chunk(1)

        def proj3(ps, sc, grp, col0, col1, xslice=None):
            """3-term hi/lo split projection: 24 DoubleRow matmuls into ps."""
            th, tl = chunks[sc]
            terms = [(w_hi, th), (w_lo, th), (w_hi, tl)]
            for t, (wt, xt) in enumerate(terms):
                for p in range(DT // 2):
                    first = (t == 0 and p == 0)
                    last = (t == len(terms) - 1 and p == DT // 2 - 1)
                    dpair = slice(2 * p, 2 * p + 2)
                    if xslice is None:
                        nc.tensor.matmul(ps, wt[:, grp, dpair, col0:col1], xt[:, dpair, :],
                                         start=first, stop=last, perf_mode=DR)
                    else:
                        nc.tensor.matmul(ps, xt[:, dpair, xslice], wt[:, grp, dpair, col0:col1],
                                         start=first, stop=last, perf_mode=DR)

        # ---- phase 1: k and v projections over this core's key half -----
        pctx = ExitStack()
        pp = pctx.enter_context(tc.tile_pool(name="pp", bufs=8, space=bass.MemorySpace.PSUM))

        # PE p-state warm-up while initial DMAs land
        jt = singles.tile([128, 512], BF16, tag="junk")
        nc.vector.memset(jt, 0.0)
        jps = pp.tile([128, 512], F32, tag="pp", name="jps")
        for _ in range(8):
            nc.tensor.matmul(jps, jt[:, 0:128], jt, start=True, stop=True)

        for sc in range(KC):
            if sc + 2 < KC:
                dma_chunk(sc + 2)
            elif sc == KC - 2:
                for g in (0,):
                    nc.sync.dma_start(out=w_hi[:, g], in_=wh[:, g])
                    nc.sync.dma_start(out=w_lo[:, g], in_=wl[:, g])
            for h in range(2):
                ps = pp.tile([128, 512], F32, tag="pp", name=f"kps{sc}{h}")
                proj3(ps, sc, 1, h * HD, (h + 1) * HD)
                nc.vector.tensor_scalar_mul(kT[:, h, sc * 512:(sc + 1) * 512], ps, INV_SC)
            for ssl in range(4):
                idx = sc * 4 + ssl
                vps = pp.tile([128, 512], F32, tag="pp", name=f"vps{idx}")
                proj3(vps[:, 0:DV], sc, 2, 0, DV,
                      xslice=slice(ssl * 128, (ssl + 1) * 128))
                nc.vector.tensor_scalar_mul(vhi[:, idx, 0:DV], vps[:, 0:DV], INV_SC)
                nc.vector.scalar_tensor_tensor(vlo[:, idx, 0:DV], vps[:, 0:DV], INV_SC,
                                               vhi[:, idx, 0:DV], MULT, SUB)
                nc.vector.tensor_scalar_mul(vaug[:, idx, 0:DV], vps[:, 0:DV], INV_SC)

        # chunk-0 q-projection at the tail of phase 1 (pp pool still open,
        # avoids a cross-pool psum wait at the phase boundary)
        for h in range(2):
            qp = pp.tile([128, 512], F32, tag="pp", name=f"qp0{h}")
            proj3(qp, 0, 0, h * HD, (h + 1) * HD)
            nc.vector.tensor_scalar_mul(qT[:, h, 0:512], qp, INV_SC)
        pctx.close()

        # ---- phase 2: per query chunk: scores, exp, PV ------------------
        ps_sc = ctx.enter_context(tc.tile_pool(name="ps_sc", bufs=2, space=bass.MemorySpace.PSUM))
        ps_pv = ctx.enter_context(tc.tile_pool(name="ps_pv", bufs=4, space=bass.MemorySpace.PSUM))

        def qproj_block(qcn, h):
            # time-shares a score-psum slot (PSUM has no spare bank)
            qp = ps_sc.tile([128, 2, 512], F32, tag="sc", name=f"qp{qcn}{h}")[:, 0, :]
            proj3(qp, qcn, 0, h * HD, (h + 1) * HD)
            nc.vector.tensor_scalar_mul(qT[:, h, qcn * 512:(qcn + 1) * 512], qp, INV_SC)

        def scores(qc, h, g):
            sg = ps_sc.tile([128, 2, 512], F32, tag="sc", name=f"sg{qc}{h}{g}")
            qTs = qT[:, h, qc * 512:(qc + 1) * 512]
            for i in range(2):
                kt = 2 * g + i
                nc.tensor.matmul(sg[:, i, :], kT[:, h, kt * 128:(kt + 1) * 128],
                                 qTs, start=True, stop=True)
            return sg

        def head_loop(qc, h, sg_first, sg_second=None, tail_filler=None):
            pv_t = ps_pv.tile([128, 4, 512], F32, tag="pv", bufs=1, name=f"pv{qc}{h}")
            pvs = [pv_t[:, i, 0:DVA] for i in range(4)]
            tail_result = [None]
            sg_prev = sg_first
            for g in range(KG):
                if g == 0 and sg_second is not None:
                    sg_next = sg_second
                else:
                    sg_next = scores(qc, h, g + 1) if g + 1 < KG else None
                if g == KG - 1 and tail_filler is not None:
                    # PE filler during the last group's exp wait
                    tail_result[0] = tail_filler()
                first, last = (g == 0), (g == KG - 1)
                if h == 0:
                    e8 = e_pool.tile([128, 2, 512], F8, tag="e8", name=f"e8_{qc}{g}")
                    nc.scalar.activation(out=e8, in_=sg_prev,
                                         func=mybir.ActivationFunctionType.Exp,
                                         bias=bias_t, scale=SCALE)
                    for qsl in range(4):
                        stat = e8[:, :, qsl * 128:(qsl + 1) * 128]
                        nc.tensor.matmul(pvs[qsl], stat, vhi[:, 2 * g:2 * g + 2, :],
                                         start=first, stop=False, perf_mode=DR)
                        nc.tensor.matmul(pvs[qsl], stat, vlo[:, 2 * g:2 * g + 2, :],
                                         start=False, stop=last, perf_mode=DR)
                else:
                    eb = e_pool.tile([128, 2, 512], BF16, tag="eb", name=f"eb_{qc}{g}")
                    nc.scalar.activation(out=eb, in_=sg_prev,
                                         func=mybir.ActivationFunctionType.Exp,
                                         bias=bias_t, scale=SCALE)
                    for qsl in range(4):
                        for i in range(2):
                            st = eb[:, i, qsl * 128:(qsl + 1) * 128]
                            nc.tensor.matmul(pvs[qsl], st, vaug[:, 2 * g + i, :],
                                             start=(first and i == 0),
                                             stop=(last and i == 1))
                sg_prev = sg_next

            onum = o_pool.tile([128, 4, DVA], BF16, tag="onum", name=f"on{qc}{h}")
            nc.vector.tensor_copy(onum, pv_t[:, :, 0:DVA])
            nc.sync.dma_start(out=out[h, :, qc * 4:qc * 4 + 4, :], in_=onum)
            return tail_result[0]

        h1_sg0 = scores(0, 0, 0)
        for qc in range(QC):
            if qc + 1 < QC and (qc + 1) not in chunks:
                dma_chunk(qc + 1)
            # h1's tail filler: next chunk's head-0 q-projection plus h2's
            # first two score groups (keeps PE busy through the last exp
            # wait and primes h2's exp chain); h2's tail filler: the next
            # chunk's first h1 score group
            def h1_tail(qc=qc):
                if qc + 1 < QC:
                    qproj_block(qc + 1, 0)
                return scores(qc, 1, 0)

            h2_sg0 = head_loop(qc, 0, h1_sg0, tail_filler=h1_tail)
            if qc + 1 < QC:
                qproj_block(qc + 1, 1)
            h1_sg0 = head_loop(qc, 1, h2_sg0,
                               tail_filler=(lambda qc=qc: scores(qc + 1, 0, 0))
                               if qc + 1 < QC else None)

    nc.compile()
    return nc


def _lam(lambda_q1, lambda_q2, lambda_k1, lambda_k2):
    lam_init = 0.8 - 0.6 * math.exp(-0.3 * DEPTH)
    l1 = math.exp(float(np.sum(lambda_q1.astype(np.float64) * lambda_k1.astype(np.float64))))
    l2 = math.exp(float(np.sum(lambda_q2.astype(np.float64) * lambda_k2.astype(np.float64))))
    return l1 + l2 + lam_init


def kernel(x, WQ, WK, WV, lambda_q1, lambda_q2, lambda_k1, lambda_k2):
    if "nc" not in _cache:
        _cache["nc"] = build_nc()
    nc = _cache["nc"]

    f8 = ml_dtypes.float8_e4m3
    lam = _lam(lambda_q1, lambda_q2, lambda_k1, lambda_k2)

    w = np.stack([np.asarray(WQ, np.float32), np.asarray(WK, np.float32),
                  np.asarray(WV, np.float32)]) * W_SC   # [3, D, 256]
    wh3 = np.asarray(w, f8)
    wl3 = np.asarray(w - wh3.astype(np.float32), f8)
    # pre-tile to [p][grp][dt][c] so weight DMAs are 4KB-contiguous runs
    tile_w = lambda a: np.ascontiguousarray(
        a.reshape(3, D // 128, 128, DV).transpose(2, 0, 1, 3).reshape(128, -1))
    wh = tile_w(wh3)
    wl = tile_w(wl3)

    in_maps = []
    for c in range(N_CORES):
        b, kh = c // 2, c % 2
        xb = np.asarray(x[b], np.float32)
        if kh:
            xb = np.concatenate([xb[SK:], xb[:SK]], axis=0)
        xbh = np.asarray(xb, f8)
        xbl = np.asarray(xb - xbh.astype(np.float32), f8)
        in_maps.append({
            "xh": np.ascontiguousarray(xbh.T),
            "xl": np.ascontiguousarray(xbl.T),
            "wh": wh, "wl": wl,
        })

    kres = None
    for attempt in range(3):
        try:
            kres = run_bass_kernel_spmd(nc, in_maps, list(range(N_CORES)))
            break
        except (ModuleNotFoundError, ImportError):
            os.environ["BASS_NEVER_TRACE"] = "1"
        except Exception:
            if attempt == 2:
                raise
            time.sleep(5)
    if kres is None:
        kres = run_bass_kernel_spmd(nc, in_maps, list(range(N_CORES)))
    _cache["last_results"] = kres
    _cache["input_names"] = ["xh", "xl", "wh", "wl"]
    res = kres.results

    out = np.empty((B, S, DV), np.float32)
    for b in range(B):
        na = res[2 * b]["out"].astype(np.float32)       # [2, S, DVA], canonical q
        nb = res[2 * b + 1]["out"].astype(np.float32)   # q rotated by SK
        nb = np.concatenate([nb[:, SK:, :], nb[:, :SK, :]], axis=1)
        n = na + nb
        o1 = n[0, :, :DV] / n[0, :, DV:DVA]
        o2 = n[1, :, :DV] / n[1, :, DV:DVA]
        out[b] = o1 - lam * o2
    return out
